# revision 6
# baseline (speedup 1.0000x reference)
"""Depthwise causal-conv1d step (single timestep) on 8 Trainium2 cores.

  out[b, h]        = sum_k w[h, k] * cat(state, x)[b, h, k] + bias[h]
  new_state[b, h,:] = cat(state, x)[b, h, 1:]

Sharding: batch dim (4096) split across 8 cores, 512 rows each; weights
replicated. Per core, batch rows sit on SBUF partitions and state is viewed
flat as [B, H*3] so every DMA is contiguous per partition.

new_state is materialized without a separate tile: the flat new_state row is
the state row shifted left by one with every 3rd slot overwritten by x.
State is loaded into a [128, 3*Hc+1] tile, x is scattered into columns 3::3
(after the k=0 product has consumed them), and columns 1:3*Hc+1 are stored.
"""

import numpy as np

import concourse.bass as bass
import concourse.tile as tile
from concourse import bacc, mybir
from concourse.bass_utils import run_bass_kernel_spmd

B = 4096
H = 4096
K = 4
NCORES = 8
BS = B // NCORES          # 512 batch rows per core
P = 128                   # SBUF partitions
NBT = BS // P             # 4 batch tiles per core
HC = 1024                 # H chunk
NHC = H // HC             # 4 chunks
SC = 3 * HC               # state columns per chunk (3072)

_cache = {}


def _build_program():
    f32 = mybir.dt.float32
    nc = bacc.Bacc("TRN2", target_bir_lowering=False, debug=False)

    x_d = nc.dram_tensor("x", [BS, H], f32, kind="ExternalInput").ap()
    st_d = nc.dram_tensor("state", [BS, 3 * H], f32, kind="ExternalInput").ap()
    wp_d = nc.dram_tensor("wp", [P, 3 * H], f32, kind="ExternalInput").ap()
    w3_d = nc.dram_tensor("w3", [P, H], f32, kind="ExternalInput").ap()
    bb_d = nc.dram_tensor("bb", [P, H], f32, kind="ExternalInput").ap()
    out_d = nc.dram_tensor("out", [BS, H], f32, kind="ExternalOutput").ap()
    ns_d = nc.dram_tensor("new_state", [BS, 3 * H], f32, kind="ExternalOutput").ap()

    with tile.TileContext(nc) as tc:
        with (
            tc.tile_pool(name="weights", bufs=1) as wpool,
            tc.tile_pool(name="xin", bufs=2) as xpool,
            tc.tile_pool(name="oacc", bufs=2) as opool,
            tc.tile_pool(name="sdata", bufs=3) as spool,
            tc.tile_pool(name="tmp", bufs=1) as tpool,
        ):
            wp = wpool.tile([P, 3 * H], f32, tag="wp")
            nc.sync.dma_start(out=wp[:], in_=wp_d[:])
            w3 = wpool.tile([P, H], f32, tag="w3")
            nc.sync.dma_start(out=w3[:], in_=w3_d[:])
            bb = wpool.tile([P, H], f32, tag="bb")
            nc.sync.dma_start(out=bb[:], in_=bb_d[:])

            for bt in range(NBT):
                r0 = bt * P
                xt = xpool.tile([P, H], f32, tag="xt")
                nc.sync.dma_start(out=xt[:], in_=x_d[r0 : r0 + P, :])
                ot = opool.tile([P, H], f32, tag="ot")
                for c in range(NHC):
                    c0 = c * SC
                    st = spool.tile([P, SC + 1], f32, tag="st")
                    nc.sync.dma_start(
                        out=st[:, 0:SC], in_=st_d[r0 : r0 + P, c0 : c0 + SC]
                    )
                    o = ot[:, c * HC : (c + 1) * HC]
                    xc = xt[:, c * HC : (c + 1) * HC]
                    tt = tpool.tile([P, HC], f32, tag="tt")
                    nc.vector.tensor_mul(o, st[:, 0:SC:3], wp[:, c0 + 0 : c0 + SC : 3])
                    nc.vector.tensor_mul(tt[:], st[:, 1 : SC + 1 : 3], wp[:, c0 + 1 : c0 + SC : 3])
                    nc.vector.tensor_add(o, o, tt[:])
                    nc.vector.tensor_mul(tt[:], st[:, 2 : SC + 1 : 3], wp[:, c0 + 2 : c0 + SC : 3])
                    nc.vector.tensor_add(o, o, tt[:])
                    nc.vector.tensor_mul(tt[:], xc, w3[:, c * HC : (c + 1) * HC])
                    nc.vector.tensor_add(o, o, tt[:])
                    nc.vector.tensor_add(o, o, bb[:, c * HC : (c + 1) * HC])
                    # shift-register tail: x becomes newest tap of new_state
                    nc.scalar.copy(st[:, 3 : SC + 1 : 3], xc)
                    nc.scalar.dma_start(
                        out=ns_d[r0 : r0 + P, c0 : c0 + SC], in_=st[:, 1 : SC + 1]
                    )
                nc.scalar.dma_start(out=out_d[r0 : r0 + P, :], in_=ot[:])

    nc.compile()
    return nc


def _get_program():
    if "nc" not in _cache:
        _cache["nc"] = _build_program()
    return _cache["nc"]


def run(x, state, weight, bias, trace=False, **spmd_kwargs):
    nc = _get_program()

    state_f = np.ascontiguousarray(state, dtype=np.float32).reshape(B, 3 * H)
    x = np.ascontiguousarray(x, dtype=np.float32)
    w = np.asarray(weight, dtype=np.float32)
    wp = np.ascontiguousarray(np.broadcast_to(w[:, 0:3].reshape(1, 3 * H), (P, 3 * H)))
    w3 = np.ascontiguousarray(np.broadcast_to(w[:, 3].reshape(1, H), (P, H)))
    bb = np.ascontiguousarray(
        np.broadcast_to(np.asarray(bias, dtype=np.float32).reshape(1, H), (P, H))
    )

    in_maps = [
        {
            "x": x[i * BS : (i + 1) * BS],
            "state": state_f[i * BS : (i + 1) * BS],
            "wp": wp,
            "w3": w3,
            "bb": bb,
        }
        for i in range(NCORES)
    ]
    res = run_bass_kernel_spmd(
        nc, in_maps, list(range(NCORES)), trace=trace, **spmd_kwargs
    )
    out = np.concatenate([res.results[i]["out"] for i in range(NCORES)], axis=0)
    new_state = np.concatenate(
        [res.results[i]["new_state"] for i in range(NCORES)], axis=0
    ).reshape(B, H, K - 1)
    return (out, new_state), res


def kernel(x, state, weight, bias):
    (out, new_state), _ = run(x, state, weight, bias, trace=False)
    return out, new_state


# revision 12
# speedup vs baseline: 1.1349x; 1.1349x over previous
"""Depthwise causal-conv1d step (single timestep) on 8 Trainium2 cores.

  out[b, h]         = sum_k w[h, k] * cat(state, x)[b, h, k] + bias[h]
  new_state[b, h, :] = cat(state, x)[b, h, 1:]

Sharding: batch dim (4096) split across 8 cores, 512 rows each; weights
replicated. Per core, batch rows sit on SBUF partitions and state is viewed
flat as [B, H*3] so every DMA is contiguous per partition.

Weights arrive as single rows ([1, N]) and are broadcast across the 128
partitions on-chip with a PE outer product (ones[128] x w_row -> PSUM),
avoiding 10 MiB of HBM traffic per core.

new_state is materialized without a separate tile: the flat new_state row is
the state row shifted left by one with every 3rd slot overwritten by x.
State is loaded into a [128, 3*Hc+1] tile, x is scattered into columns 3::3
(after the interleaved product pass has consumed them), and columns
1:3*Hc+1 are stored.
"""

import numpy as np

import concourse.bass as bass
import concourse.tile as tile
from concourse import bacc, mybir
from concourse.bass_utils import run_bass_kernel_spmd

B = 4096
H = 4096
K = 4
NCORES = 8
BS = B // NCORES          # 512 batch rows per core
P = 128                   # SBUF partitions
NBT = BS // P             # 4 batch tiles per core
HC = 1024                 # H chunk
NHC = H // HC             # 4 chunks
SC = 3 * HC               # state columns per chunk (3072)
MMN = 512                 # PSUM free-dim per broadcast matmul

_cache = {}


def _build_program():
    f32 = mybir.dt.float32
    nc = bacc.Bacc("TRN2", target_bir_lowering=False, debug=False)

    x_d = nc.dram_tensor("x", [BS, H], f32, kind="ExternalInput").ap()
    st_d = nc.dram_tensor("state", [BS, 3 * H], f32, kind="ExternalInput").ap()
    # weight rows: interleaved taps 0..2 (w[h,k] at 3h+k), tap 3, bias
    wp_d = nc.dram_tensor("wp", [1, 3 * H], f32, kind="ExternalInput").ap()
    w3_d = nc.dram_tensor("w3", [1, H], f32, kind="ExternalInput").ap()
    bb_d = nc.dram_tensor("bb", [1, H], f32, kind="ExternalInput").ap()
    out_d = nc.dram_tensor("out", [BS, H], f32, kind="ExternalOutput").ap()
    ns_d = nc.dram_tensor("new_state", [BS, 3 * H], f32, kind="ExternalOutput").ap()

    with tile.TileContext(nc) as tc:
        with (
            tc.tile_pool(name="weights", bufs=1) as wpool,
            tc.tile_pool(name="wrows", bufs=2) as rpool,
            tc.tile_pool(name="psum", bufs=4, space="PSUM") as ppool,
            tc.tile_pool(name="xin", bufs=2) as xpool,
            tc.tile_pool(name="oacc", bufs=2) as opool,
            tc.tile_pool(name="sdata", bufs=3) as spool,
            tc.tile_pool(name="tprod", bufs=1) as tpool,
        ):
            # --- one-time: broadcast weight rows to all 128 partitions ---
            ones = rpool.tile([1, P], f32, tag="ones")
            nc.vector.memset(ones[:], 1.0)
            STRIP = 2048  # strip of the concatenated [wp | w3 | bb] row
            rows = [(wp_d, 3 * H), (w3_d, H), (bb_d, H)]

            wall = wpool.tile([P, 5 * H], f32, tag="wall")
            off = 0
            for row_d, n in rows:
                for s0 in range(0, n, STRIP):
                    wrow = rpool.tile([1, STRIP], f32, tag="wrow")
                    nc.sync.dma_start(out=wrow[:], in_=row_d[:, s0 : s0 + STRIP])
                    for m0 in range(0, STRIP, MMN):
                        pt = ppool.tile([P, MMN], f32, tag="pt")
                        nc.tensor.matmul(
                            pt[:],
                            ones[:],
                            wrow[:, m0 : m0 + MMN],
                            start=True,
                            stop=True,
                        )
                        nc.scalar.copy(
                            wall[:, off + s0 + m0 : off + s0 + m0 + MMN], pt[:]
                        )
                off += n
            wp = wall[:, 0 : 3 * H]
            w3 = wall[:, 3 * H : 4 * H]
            bb = wall[:, 4 * H : 5 * H]

            # --- main loop ---
            for bt in range(NBT):
                r0 = bt * P
                xt = xpool.tile([P, H], f32, tag="xt")
                nc.sync.dma_start(out=xt[:], in_=x_d[r0 : r0 + P, :])
                for c in range(NHC):
                    c0 = c * SC
                    st = spool.tile([P, SC + 1], f32, tag="st")
                    nc.sync.dma_start(
                        out=st[:, 0:SC], in_=st_d[r0 : r0 + P, c0 : c0 + SC]
                    )
                    xc = xt[:, c * HC : (c + 1) * HC]
                    tp = tpool.tile([P, SC], f32, tag="tp")
                    ot = opool.tile([P, HC], f32, tag="ot")
                    # interleaved product pass, then pairwise tap reduction
                    nc.vector.tensor_mul(tp[:], st[:, 0:SC], wp[:, c0 : c0 + SC])
                    nc.vector.tensor_add(ot[:], tp[:, 0:SC:3], tp[:, 1:SC:3])
                    nc.vector.tensor_add(ot[:], ot[:], tp[:, 2:SC:3])
                    tx = tp[:, 0:HC]
                    nc.vector.tensor_mul(tx, xc, w3[:, c * HC : (c + 1) * HC])
                    nc.vector.tensor_add(ot[:], ot[:], tx)
                    nc.vector.tensor_add(ot[:], ot[:], bb[:, c * HC : (c + 1) * HC])
                    # shift-register tail: x becomes newest tap of new_state
                    nc.scalar.copy(st[:, 3 : SC + 1 : 3], xc)
                    nc.scalar.dma_start(
                        out=ns_d[r0 : r0 + P, c0 : c0 + SC], in_=st[:, 1 : SC + 1]
                    )
                    nc.scalar.dma_start(
                        out=out_d[r0 : r0 + P, c * HC : (c + 1) * HC], in_=ot[:]
                    )

    nc.compile()
    return nc


def _get_program():
    if "nc" not in _cache:
        _cache["nc"] = _build_program()
    return _cache["nc"]


def run(x, state, weight, bias, trace=False, **spmd_kwargs):
    nc = _get_program()

    state_f = np.ascontiguousarray(state, dtype=np.float32).reshape(B, 3 * H)
    x = np.ascontiguousarray(x, dtype=np.float32)
    w = np.asarray(weight, dtype=np.float32)
    wp = np.ascontiguousarray(w[:, 0:3]).reshape(1, 3 * H)
    w3 = np.ascontiguousarray(w[:, 3]).reshape(1, H)
    bb = np.ascontiguousarray(np.asarray(bias, dtype=np.float32)).reshape(1, H)

    in_maps = [
        {
            "x": x[i * BS : (i + 1) * BS],
            "state": state_f[i * BS : (i + 1) * BS],
            "wp": wp,
            "w3": w3,
            "bb": bb,
        }
        for i in range(NCORES)
    ]
    res = run_bass_kernel_spmd(
        nc, in_maps, list(range(NCORES)), trace=trace, **spmd_kwargs
    )
    out = np.concatenate([res.results[i]["out"] for i in range(NCORES)], axis=0)
    new_state = np.concatenate(
        [res.results[i]["new_state"] for i in range(NCORES)], axis=0
    ).reshape(B, H, K - 1)
    return (out, new_state), res


def kernel(x, state, weight, bias):
    (out, new_state), _ = run(x, state, weight, bias, trace=False)
    return out, new_state


# revision 16
# speedup vs baseline: 1.1563x; 1.0189x over previous
"""Depthwise causal-conv1d step (single timestep) on 8 Trainium2 cores.

  out[b, h]         = sum_k w[h, k] * cat(state, x)[b, h, k] + bias[h]
  new_state[b, h, :] = cat(state, x)[b, h, 1:]

Sharding: batch dim (4096) split across 8 cores, 512 rows each; weights
replicated. Per core, batch rows sit on SBUF partitions and state is viewed
flat as [B, H*3] so every DMA is contiguous per partition.

Weights arrive as one concatenated row (per H-chunk [w3_c | taps interleaved],
then bias) and are broadcast across the 128 partitions on-chip with one-hot
PE matmuls (PSUM) evacuated just-in-time ahead of their consuming chunk.

Each H-chunk tile holds [x_c | state_c | spare]; one contiguous product pass
multiplies both the three taps and the x term by their weights, then four
adds reduce taps + x-term + bias. new_state needs no separate tile: the flat
new_state row is the state row shifted left by one with every 3rd slot
overwritten by x, so x is scattered into the state region (after the product
pass consumed it) and a shifted contiguous view is stored.
"""

import numpy as np

import concourse.bass as bass
import concourse.tile as tile
from concourse import bacc, mybir
from concourse.bass_utils import run_bass_kernel_spmd

B = 4096
H = 4096
K = 4
NCORES = 8
BS = B // NCORES          # 512 batch rows per core
P = 128                   # SBUF partitions
NBT = BS // P             # 4 batch tiles per core
HC = 1024                 # H chunk
NHC = H // HC             # 4 chunks
SC = 3 * HC               # state columns per chunk (3072)
CW = HC + SC              # per-chunk weight/product width (4096)
WTOT = NHC * CW + H       # wall columns: per-chunk blocks + bias (20480)
MMN = 512                 # PSUM free-dim per broadcast matmul
WR = 8                    # weight-row partitions
WC = WTOT // WR           # 2560 cols per weight row

_cache = {}


def _build_program():
    f32 = mybir.dt.float32
    nc = bacc.Bacc("TRN2", target_bir_lowering=False, debug=False)

    x_d = nc.dram_tensor("x", [BS, H], f32, kind="ExternalInput").ap()
    st_d = nc.dram_tensor("state", [BS, 3 * H], f32, kind="ExternalInput").ap()
    w_d = nc.dram_tensor("wrow", [1, WTOT], f32, kind="ExternalInput").ap()
    oh_d = nc.dram_tensor("onehot", [WR, WR * P], f32, kind="ExternalInput").ap()
    out_d = nc.dram_tensor("out", [BS, H], f32, kind="ExternalOutput").ap()
    ns_d = nc.dram_tensor("new_state", [BS, 3 * H], f32, kind="ExternalOutput").ap()

    with tile.TileContext(nc) as tc:
        with (
            tc.tile_pool(name="weights", bufs=1) as wpool,
            tc.tile_pool(name="wrows", bufs=1) as rpool,
            tc.tile_pool(name="psum", bufs=4, space="PSUM") as ppool,
            tc.tile_pool(name="oacc", bufs=3) as opool,
            tc.tile_pool(name="sdata", bufs=4) as spool,
            tc.tile_pool(name="tprod", bufs=1) as tpool,
        ):
            # one-hot selector: oh[k, 128*r + m] = (k == r)
            oh = rpool.tile([WR, WR * P], f32, tag="oh")
            nc.sync.dma_start(out=oh[:], in_=oh_d[:])
            wrow = rpool.tile([WR, WC], f32, tag="wrow")
            nc.sync.dma_start(
                out=wrow[:], in_=w_d.rearrange("o (a b) -> (o a) b", b=WC)
            )
            wall = wpool.tile([P, WTOT], f32, tag="wall")

            def bcast_strip(s):
                """Broadcast wall cols [512*s, 512*s+512) from the weight rows."""
                r, cblk = divmod(s, WC // MMN)
                pt = ppool.tile([P, MMN], f32, tag="pt")
                nc.tensor.matmul(
                    pt[:],
                    oh[:, r * P : (r + 1) * P],
                    wrow[:, cblk * MMN : (cblk + 1) * MMN],
                    start=True,
                    stop=True,
                )
                nc.scalar.copy(wall[:, s * MMN : (s + 1) * MMN], pt[:])

            for bt in range(NBT):
                r0 = bt * P
                for c in range(NHC):
                    if bt == 0:
                        # weights this chunk reads: block c and its bias slice
                        for s in range(c * CW // MMN, (c + 1) * CW // MMN):
                            bcast_strip(s)
                        for s in range(
                            (NHC * CW + c * HC) // MMN,
                            (NHC * CW + (c + 1) * HC) // MMN,
                        ):
                            bcast_strip(s)
                    st = spool.tile([P, CW + 1], f32, tag="st")
                    nc.sync.dma_start(
                        out=st[:, 0:HC], in_=x_d[r0 : r0 + P, c * HC : (c + 1) * HC]
                    )
                    nc.sync.dma_start(
                        out=st[:, HC:CW], in_=st_d[r0 : r0 + P, c * SC : (c + 1) * SC]
                    )
                    tp = tpool.tile([P, CW], f32, tag="tp")
                    ot = opool.tile([P, HC], f32, tag="ot")
                    wc = c * CW
                    nc.vector.tensor_mul(tp[:], st[:, 0:CW], wall[:, wc : wc + CW])
                    nc.vector.tensor_add(ot[:], tp[:, HC:CW:3], tp[:, HC + 1 : CW : 3])
                    nc.vector.tensor_add(ot[:], ot[:], tp[:, HC + 2 : CW : 3])
                    nc.vector.tensor_add(ot[:], ot[:], tp[:, 0:HC])
                    bc = NHC * CW + c * HC
                    nc.vector.tensor_add(ot[:], ot[:], wall[:, bc : bc + HC])
                    # shift-register tail: x becomes newest tap of new_state
                    nc.scalar.copy(st[:, HC + 3 : CW + 1 : 3], st[:, 0:HC])
                    nc.scalar.dma_start(
                        out=ns_d[r0 : r0 + P, c * SC : (c + 1) * SC],
                        in_=st[:, HC + 1 : CW + 1],
                    )
                    nc.scalar.dma_start(
                        out=out_d[r0 : r0 + P, c * HC : (c + 1) * HC], in_=ot[:]
                    )

    nc.compile()
    return nc


def _get_program():
    if "nc" not in _cache:
        _cache["nc"] = _build_program()
    return _cache["nc"]


def _pack_weights(weight, bias):
    w = np.asarray(weight, dtype=np.float32)
    row = np.empty(WTOT, dtype=np.float32)
    for c in range(NHC):
        h0 = c * HC
        row[c * CW : c * CW + HC] = w[h0 : h0 + HC, 3]
        row[c * CW + HC : (c + 1) * CW] = w[h0 : h0 + HC, 0:3].reshape(-1)
    row[NHC * CW :] = np.asarray(bias, dtype=np.float32)
    return row.reshape(1, WTOT)


def run(x, state, weight, bias, trace=False, **spmd_kwargs):
    nc = _get_program()

    state_f = np.ascontiguousarray(state, dtype=np.float32).reshape(B, 3 * H)
    x = np.ascontiguousarray(x, dtype=np.float32)
    wrow = _pack_weights(weight, bias)
    onehot = np.zeros((WR, WR * P), dtype=np.float32)
    for r in range(WR):
        onehot[r, r * P : (r + 1) * P] = 1.0

    in_maps = [
        {
            "x": x[i * BS : (i + 1) * BS],
            "state": state_f[i * BS : (i + 1) * BS],
            "wrow": wrow,
            "onehot": onehot,
        }
        for i in range(NCORES)
    ]
    res = run_bass_kernel_spmd(
        nc, in_maps, list(range(NCORES)), trace=trace, **spmd_kwargs
    )
    out = np.concatenate([res.results[i]["out"] for i in range(NCORES)], axis=0)
    new_state = np.concatenate(
        [res.results[i]["new_state"] for i in range(NCORES)], axis=0
    ).reshape(B, H, K - 1)
    return (out, new_state), res


def kernel(x, state, weight, bias):
    (out, new_state), _ = run(x, state, weight, bias, trace=False)
    return out, new_state


# revision 18
# speedup vs baseline: 1.1567x; 1.0004x over previous
"""Depthwise causal-conv1d step (single timestep) on 8 Trainium2 cores.

  out[b, h]         = sum_k w[h, k] * cat(state, x)[b, h, k] + bias[h]
  new_state[b, h, :] = cat(state, x)[b, h, 1:]

Sharding: batch dim (4096) split across 8 cores, 512 rows each; weights
replicated. Per core, batch rows sit on SBUF partitions and state is viewed
flat as [B, H*3] so every DMA is contiguous per partition.

Weights arrive as one concatenated row (per H-chunk [w3_c | taps interleaved],
then bias) and are broadcast across the 128 partitions on-chip with one-hot
PE matmuls (PSUM) evacuated just-in-time ahead of their consuming chunk.

Each H-chunk tile holds [x_c | state_c | spare]; one contiguous product pass
multiplies both the three taps and the x term by their weights, then four
adds reduce taps + x-term + bias. new_state needs no separate tile: the flat
new_state row is the state row shifted left by one with every 3rd slot
overwritten by x, so x is scattered into the state region (after the product
pass consumed it) and a shifted contiguous view is stored.
"""

import numpy as np

import concourse.bass as bass
import concourse.tile as tile
from concourse import bacc, mybir
from concourse.bass_utils import run_bass_kernel_spmd

B = 4096
H = 4096
K = 4
NCORES = 8
BS = B // NCORES          # 512 batch rows per core
P = 128                   # SBUF partitions
NBT = BS // P             # 4 batch tiles per core
HC = 1024                 # H chunk
NHC = H // HC             # 4 chunks
SC = 3 * HC               # state columns per chunk (3072)
CW = HC + SC              # per-chunk weight/product width (4096)
WTOT = NHC * CW + H       # wall columns: per-chunk blocks + bias (20480)
MMN = 512                 # PSUM free-dim per broadcast matmul
WR = 8                    # weight-row partitions
WC = WTOT // WR           # 2560 cols per weight row

_cache = {}


def _build_program():
    f32 = mybir.dt.float32
    nc = bacc.Bacc("TRN2", target_bir_lowering=False, debug=False)

    x_d = nc.dram_tensor("x", [BS, H], f32, kind="ExternalInput").ap()
    st_d = nc.dram_tensor("state", [BS, 3 * H], f32, kind="ExternalInput").ap()
    w_d = nc.dram_tensor("wrow", [1, WTOT], f32, kind="ExternalInput").ap()
    oh_d = nc.dram_tensor("onehot", [WR, WR * P], f32, kind="ExternalInput").ap()
    out_d = nc.dram_tensor("out", [BS, H], f32, kind="ExternalOutput").ap()
    ns_d = nc.dram_tensor("new_state", [BS, 3 * H], f32, kind="ExternalOutput").ap()

    with tile.TileContext(nc) as tc:
        with (
            tc.tile_pool(name="weights", bufs=1) as wpool,
            tc.tile_pool(name="wrows", bufs=1) as rpool,
            tc.tile_pool(name="psum", bufs=4, space="PSUM") as ppool,
            tc.tile_pool(name="oacc", bufs=3) as opool,
            tc.tile_pool(name="sdata", bufs=4) as spool,
            tc.tile_pool(name="tprod", bufs=1) as tpool,
        ):
            # one-hot selector: oh[k, 128*r + m] = (k == r)
            oh = rpool.tile([WR, WR * P], f32, tag="oh")
            nc.sync.dma_start(out=oh[:], in_=oh_d[:])
            wrow = rpool.tile([WR, WC], f32, tag="wrow")
            nc.sync.dma_start(
                out=wrow[:], in_=w_d.rearrange("o (a b) -> (o a) b", b=WC)
            )
            wall = wpool.tile([P, WTOT], f32, tag="wall")

            def bcast_unit(u):
                """Broadcast wall cols [1024*u, 1024*u+1024) from the weight rows."""
                pt = ppool.tile([P, 2 * MMN], f32, tag="pt")
                for i in range(2):
                    s = 2 * u + i
                    r, cblk = divmod(s, WC // MMN)
                    nc.tensor.matmul(
                        pt[:, i * MMN : (i + 1) * MMN],
                        oh[:, r * P : (r + 1) * P],
                        wrow[:, cblk * MMN : (cblk + 1) * MMN],
                        start=True,
                        stop=True,
                    )
                nc.scalar.copy(wall[:, 2 * u * MMN : 2 * (u + 1) * MMN], pt[:])

            for bt in range(NBT):
                r0 = bt * P
                for c in range(NHC):
                    if bt == 0:
                        # weights this chunk reads: block c and its bias slice
                        for u in range(c * CW // HC, (c + 1) * CW // HC):
                            bcast_unit(u)
                        bcast_unit((NHC * CW + c * HC) // HC)
                    st = spool.tile([P, CW + 1], f32, tag="st")
                    nc.sync.dma_start(
                        out=st[:, 0:HC], in_=x_d[r0 : r0 + P, c * HC : (c + 1) * HC]
                    )
                    nc.sync.dma_start(
                        out=st[:, HC:CW], in_=st_d[r0 : r0 + P, c * SC : (c + 1) * SC]
                    )
                    tp = tpool.tile([P, CW], f32, tag="tp")
                    ot = opool.tile([P, HC], f32, tag="ot")
                    wc = c * CW
                    nc.vector.tensor_mul(tp[:], st[:, 0:CW], wall[:, wc : wc + CW])
                    nc.vector.tensor_add(ot[:], tp[:, HC:CW:3], tp[:, HC + 1 : CW : 3])
                    nc.vector.tensor_add(ot[:], ot[:], tp[:, HC + 2 : CW : 3])
                    nc.vector.tensor_add(ot[:], ot[:], tp[:, 0:HC])
                    bc = NHC * CW + c * HC
                    nc.vector.tensor_add(ot[:], ot[:], wall[:, bc : bc + HC])
                    # shift-register tail: x becomes newest tap of new_state
                    nc.scalar.copy(st[:, HC + 3 : CW + 1 : 3], st[:, 0:HC])
                    nc.scalar.dma_start(
                        out=ns_d[r0 : r0 + P, c * SC : (c + 1) * SC],
                        in_=st[:, HC + 1 : CW + 1],
                    )
                    nc.scalar.dma_start(
                        out=out_d[r0 : r0 + P, c * HC : (c + 1) * HC], in_=ot[:]
                    )

    nc.compile()
    return nc


def _get_program():
    if "nc" not in _cache:
        _cache["nc"] = _build_program()
    return _cache["nc"]


def _pack_weights(weight, bias):
    w = np.asarray(weight, dtype=np.float32)
    row = np.empty(WTOT, dtype=np.float32)
    for c in range(NHC):
        h0 = c * HC
        row[c * CW : c * CW + HC] = w[h0 : h0 + HC, 3]
        row[c * CW + HC : (c + 1) * CW] = w[h0 : h0 + HC, 0:3].reshape(-1)
    row[NHC * CW :] = np.asarray(bias, dtype=np.float32)
    return row.reshape(1, WTOT)


def run(x, state, weight, bias, trace=False, **spmd_kwargs):
    nc = _get_program()

    state_f = np.ascontiguousarray(state, dtype=np.float32).reshape(B, 3 * H)
    x = np.ascontiguousarray(x, dtype=np.float32)
    wrow = _pack_weights(weight, bias)
    onehot = np.zeros((WR, WR * P), dtype=np.float32)
    for r in range(WR):
        onehot[r, r * P : (r + 1) * P] = 1.0

    in_maps = [
        {
            "x": x[i * BS : (i + 1) * BS],
            "state": state_f[i * BS : (i + 1) * BS],
            "wrow": wrow,
            "onehot": onehot,
        }
        for i in range(NCORES)
    ]
    res = run_bass_kernel_spmd(
        nc, in_maps, list(range(NCORES)), trace=trace, **spmd_kwargs
    )
    out = np.concatenate([res.results[i]["out"] for i in range(NCORES)], axis=0)
    new_state = np.concatenate(
        [res.results[i]["new_state"] for i in range(NCORES)], axis=0
    ).reshape(B, H, K - 1)
    return (out, new_state), res


def kernel(x, state, weight, bias):
    (out, new_state), _ = run(x, state, weight, bias, trace=False)
    return out, new_state


# revision 19
# speedup vs baseline: 1.1949x; 1.0330x over previous
"""Depthwise causal-conv1d step (single timestep) on 8 Trainium2 cores.

  out[b, h]         = sum_k w[h, k] * cat(state, x)[b, h, k] + bias[h]
  new_state[b, h, :] = cat(state, x)[b, h, 1:]

Sharding: batch dim (4096) split across 8 cores, 512 rows each; weights
replicated. Per core, batch rows sit on SBUF partitions and state is viewed
flat as [B, H*3] so every DMA is contiguous per partition.

Weights arrive as one concatenated row (per H-chunk [w3_c | taps interleaved],
then bias) and are broadcast across the 128 partitions on-chip with one-hot
PE matmuls (PSUM) evacuated just-in-time ahead of their consuming chunk.

Each H-chunk tile holds [x_c | state_c | spare]; one contiguous product pass
multiplies both the three taps and the x term by their weights, then four
adds reduce taps + x-term + bias. new_state needs no separate tile: the flat
new_state row is the state row shifted left by one with every 3rd slot
overwritten by x, so x is scattered into the state region (after the product
pass consumed it) and a shifted contiguous view is stored.
"""

import numpy as np

import concourse.bass as bass
import concourse.tile as tile
from concourse import bacc, mybir
from concourse.bass_utils import run_bass_kernel_spmd

B = 4096
H = 4096
K = 4
NCORES = 8
BS = B // NCORES          # 512 batch rows per core
P = 128                   # SBUF partitions
NBT = BS // P             # 4 batch tiles per core
HC = 1024                 # H chunk
NHC = H // HC             # 4 chunks
SC = 3 * HC               # state columns per chunk (3072)
CW = HC + SC              # per-chunk weight/product width (4096)
WTOT = NHC * CW + H       # wall columns: per-chunk blocks + bias (20480)
MMN = 512                 # PSUM free-dim per broadcast matmul
WR = 8                    # weight-row partitions
WC = WTOT // WR           # 2560 cols per weight row

_cache = {}


def _build_program():
    f32 = mybir.dt.float32
    nc = bacc.Bacc("TRN2", target_bir_lowering=False, debug=False)

    x_d = nc.dram_tensor("x", [BS, H], f32, kind="ExternalInput").ap()
    st_d = nc.dram_tensor("state", [BS, 3 * H], f32, kind="ExternalInput").ap()
    w_d = nc.dram_tensor("wrow", [1, WTOT], f32, kind="ExternalInput").ap()
    oh_d = nc.dram_tensor("onehot", [WR, WR * P], f32, kind="ExternalInput").ap()
    out_d = nc.dram_tensor("out", [BS, H], f32, kind="ExternalOutput").ap()
    ns_d = nc.dram_tensor("new_state", [BS, 3 * H], f32, kind="ExternalOutput").ap()

    with tile.TileContext(nc) as tc:
        with (
            tc.tile_pool(name="weights", bufs=1) as wpool,
            tc.tile_pool(name="wrows", bufs=1) as rpool,
            tc.tile_pool(name="psum", bufs=4, space="PSUM") as ppool,
            tc.tile_pool(name="oacc", bufs=3) as opool,
            tc.tile_pool(name="sdata", bufs=4) as spool,
            tc.tile_pool(name="tprod", bufs=1) as tpool,
        ):
            # one-hot selector: oh[k, 128*r + m] = (k == r)
            oh = rpool.tile([WR, WR * P], f32, tag="oh")
            nc.sync.dma_start(out=oh[:], in_=oh_d[:])
            wrow = rpool.tile([WR, WC], f32, tag="wrow")
            nc.sync.dma_start(
                out=wrow[:], in_=w_d.rearrange("o (a b) -> (o a) b", b=WC)
            )
            wall = wpool.tile([P, WTOT], f32, tag="wall")

            def bcast_unit(u):
                """Broadcast wall cols [1024*u, 1024*u+1024) from the weight rows."""
                pt = ppool.tile([P, 2 * MMN], f32, tag="pt")
                for i in range(2):
                    s = 2 * u + i
                    r, cblk = divmod(s, WC // MMN)
                    nc.tensor.matmul(
                        pt[:, i * MMN : (i + 1) * MMN],
                        oh[:, r * P : (r + 1) * P],
                        wrow[:, cblk * MMN : (cblk + 1) * MMN],
                        start=True,
                        stop=True,
                    )
                nc.scalar.copy(wall[:, 2 * u * MMN : 2 * (u + 1) * MMN], pt[:])

            # All broadcast units up front, ordered by consuming chunk, so no
            # later cross-engine store-wait on the in-order ACT queue can
            # delay a weight evacuation.
            for c in range(NHC):
                for u in range(c * CW // HC, (c + 1) * CW // HC):
                    bcast_unit(u)
                bcast_unit((NHC * CW + c * HC) // HC)

            for bt in range(NBT):
                r0 = bt * P
                for c in range(NHC):
                    st = spool.tile([P, CW + 1], f32, tag="st")
                    nc.sync.dma_start(
                        out=st[:, 0:HC], in_=x_d[r0 : r0 + P, c * HC : (c + 1) * HC]
                    )
                    nc.sync.dma_start(
                        out=st[:, HC:CW], in_=st_d[r0 : r0 + P, c * SC : (c + 1) * SC]
                    )
                    tp = tpool.tile([P, CW], f32, tag="tp")
                    ot = opool.tile([P, HC], f32, tag="ot")
                    wc = c * CW
                    nc.vector.tensor_mul(tp[:], st[:, 0:CW], wall[:, wc : wc + CW])
                    nc.vector.tensor_add(ot[:], tp[:, HC:CW:3], tp[:, HC + 1 : CW : 3])
                    nc.vector.tensor_add(ot[:], ot[:], tp[:, HC + 2 : CW : 3])
                    nc.vector.tensor_add(ot[:], ot[:], tp[:, 0:HC])
                    bc = NHC * CW + c * HC
                    nc.vector.tensor_add(ot[:], ot[:], wall[:, bc : bc + HC])
                    # shift-register tail: x becomes newest tap of new_state
                    nc.scalar.copy(st[:, HC + 3 : CW + 1 : 3], st[:, 0:HC])
                    nc.scalar.dma_start(
                        out=ns_d[r0 : r0 + P, c * SC : (c + 1) * SC],
                        in_=st[:, HC + 1 : CW + 1],
                    )
                    nc.scalar.dma_start(
                        out=out_d[r0 : r0 + P, c * HC : (c + 1) * HC], in_=ot[:]
                    )

    nc.compile()
    return nc


def _get_program():
    if "nc" not in _cache:
        _cache["nc"] = _build_program()
    return _cache["nc"]


def _pack_weights(weight, bias):
    w = np.asarray(weight, dtype=np.float32)
    row = np.empty(WTOT, dtype=np.float32)
    for c in range(NHC):
        h0 = c * HC
        row[c * CW : c * CW + HC] = w[h0 : h0 + HC, 3]
        row[c * CW + HC : (c + 1) * CW] = w[h0 : h0 + HC, 0:3].reshape(-1)
    row[NHC * CW :] = np.asarray(bias, dtype=np.float32)
    return row.reshape(1, WTOT)


def run(x, state, weight, bias, trace=False, **spmd_kwargs):
    nc = _get_program()

    state_f = np.ascontiguousarray(state, dtype=np.float32).reshape(B, 3 * H)
    x = np.ascontiguousarray(x, dtype=np.float32)
    wrow = _pack_weights(weight, bias)
    onehot = np.zeros((WR, WR * P), dtype=np.float32)
    for r in range(WR):
        onehot[r, r * P : (r + 1) * P] = 1.0

    in_maps = [
        {
            "x": x[i * BS : (i + 1) * BS],
            "state": state_f[i * BS : (i + 1) * BS],
            "wrow": wrow,
            "onehot": onehot,
        }
        for i in range(NCORES)
    ]
    res = run_bass_kernel_spmd(
        nc, in_maps, list(range(NCORES)), trace=trace, **spmd_kwargs
    )
    out = np.concatenate([res.results[i]["out"] for i in range(NCORES)], axis=0)
    new_state = np.concatenate(
        [res.results[i]["new_state"] for i in range(NCORES)], axis=0
    ).reshape(B, H, K - 1)
    return (out, new_state), res


def kernel(x, state, weight, bias):
    (out, new_state), _ = run(x, state, weight, bias, trace=False)
    return out, new_state


# revision 22
# speedup vs baseline: 1.3331x; 1.1157x over previous
"""Depthwise causal-conv1d step (single timestep) on 8 Trainium2 cores.

  out[b, h]         = sum_k w[h, k] * cat(state, x)[b, h, k] + bias[h]
  new_state[b, h, :] = cat(state, x)[b, h, 1:]

Sharding: batch dim (4096) split across 8 cores, 512 rows each; weights
replicated. Per core, batch rows sit on SBUF partitions and state is viewed
flat as [B, H*3] so every DMA is contiguous per partition.

Weights arrive as one concatenated row (per H-chunk [w3_c | taps interleaved],
then bias) and are broadcast across the 128 partitions on-chip with one-hot
PE matmuls (PSUM) evacuated just-in-time ahead of their consuming chunk.

Each H-chunk tile holds [x_c | state_c | spare]; one contiguous product pass
multiplies both the three taps and the x term by their weights, then four
adds reduce taps + x-term + bias. new_state needs no separate tile: the flat
new_state row is the state row shifted left by one with every 3rd slot
overwritten by x, so x is scattered into the state region (after the product
pass consumed it) and a shifted contiguous view is stored.
"""

import numpy as np

import concourse.bass as bass
import concourse.tile as tile
from concourse import bacc, mybir
from concourse.bass_utils import run_bass_kernel_spmd

B = 4096
H = 4096
K = 4
NCORES = 8
BS = B // NCORES          # 512 batch rows per core
P = 128                   # SBUF partitions
NBT = BS // P             # 4 batch tiles per core
HC = 1024                 # H chunk
NHC = H // HC             # 4 chunks
SC = 3 * HC               # state columns per chunk (3072)
CW = HC + SC              # per-chunk weight/product width (4096)
WTOT = NHC * CW + H       # wall columns: per-chunk blocks + bias (20480)
MMN = 512                 # PSUM free-dim per broadcast matmul
WR = 8                    # weight-row partitions
WC = WTOT // WR           # 2560 cols per weight row

_cache = {}


def _build_program():
    f32 = mybir.dt.float32
    nc = bacc.Bacc("TRN2", target_bir_lowering=False, debug=False)

    x_d = nc.dram_tensor("x", [BS, H], f32, kind="ExternalInput").ap()
    st_d = nc.dram_tensor("state", [BS, 3 * H], f32, kind="ExternalInput").ap()
    w_d = nc.dram_tensor("wrow", [1, WTOT], f32, kind="ExternalInput").ap()
    oh_d = nc.dram_tensor("onehot", [WR, WR * P], f32, kind="ExternalInput").ap()
    out_d = nc.dram_tensor("out", [BS, H], f32, kind="ExternalOutput").ap()
    ns_d = nc.dram_tensor("new_state", [BS, 3 * H], f32, kind="ExternalOutput").ap()

    with tile.TileContext(nc) as tc:
        with (
            tc.tile_pool(name="weights", bufs=1) as wpool,
            tc.tile_pool(name="wrows", bufs=1) as rpool,
            tc.tile_pool(name="psum", bufs=4, space="PSUM") as ppool,
            tc.tile_pool(name="oacc", bufs=3) as opool,
            tc.tile_pool(name="sdata", bufs=4) as spool,
            tc.tile_pool(name="tprod", bufs=1) as tpool,
        ):
            # one-hot selector: oh[k, 128*r + m] = (k == r)
            oh = rpool.tile([WR, WR * P], f32, tag="oh")
            nc.sync.dma_start(out=oh[:], in_=oh_d[:])
            wrow = rpool.tile([WR, WC], f32, tag="wrow")
            nc.sync.dma_start(
                out=wrow[:], in_=w_d.rearrange("o (a b) -> (o a) b", b=WC)
            )
            wall = wpool.tile([P, WTOT], f32, tag="wall")

            def bcast_unit(u):
                """Broadcast wall cols [1024*u, 1024*u+1024) from the weight rows."""
                pt = ppool.tile([P, 2 * MMN], f32, tag="pt")
                for i in range(2):
                    s = 2 * u + i
                    r, cblk = divmod(s, WC // MMN)
                    nc.tensor.matmul(
                        pt[:, i * MMN : (i + 1) * MMN],
                        oh[:, r * P : (r + 1) * P],
                        wrow[:, cblk * MMN : (cblk + 1) * MMN],
                        start=True,
                        stop=True,
                    )
                nc.scalar.copy(wall[:, 2 * u * MMN : 2 * (u + 1) * MMN], pt[:])

            def bcast_chunk(c):
                """Queue the broadcast of chunk c's weight block + bias slice."""
                for u in range(c * CW // HC, (c + 1) * CW // HC):
                    bcast_unit(u)
                bcast_unit((NHC * CW + c * HC) // HC)

            bcast_chunk(0)
            # chunk-outer: the first four iterations need only chunk-0's
            # weights; chunk c+1's weights are evacuated during chunk c's
            # first iteration (after the ns-store issue, before the
            # out-store's cross-engine wait can block the in-order ACT
            # queue), giving a full batch-sweep of lead time.
            for c in range(NHC):
                for bt in range(NBT):
                    r0 = bt * P
                    st = spool.tile([P, CW + 1], f32, tag="st")
                    nc.sync.dma_start(
                        out=st[:, 0:HC], in_=x_d[r0 : r0 + P, c * HC : (c + 1) * HC]
                    )
                    nc.sync.dma_start(
                        out=st[:, HC:CW], in_=st_d[r0 : r0 + P, c * SC : (c + 1) * SC]
                    )
                    tp = tpool.tile([P, CW], f32, tag="tp")
                    ot = opool.tile([P, HC], f32, tag="ot")
                    wc = c * CW
                    nc.vector.tensor_mul(tp[:], st[:, 0:CW], wall[:, wc : wc + CW])
                    nc.vector.tensor_add(ot[:], tp[:, HC:CW:3], tp[:, HC + 1 : CW : 3])
                    nc.vector.tensor_add(ot[:], ot[:], tp[:, HC + 2 : CW : 3])
                    nc.vector.tensor_add(ot[:], ot[:], tp[:, 0:HC])
                    bc = NHC * CW + c * HC
                    nc.vector.tensor_add(ot[:], ot[:], wall[:, bc : bc + HC])
                    # shift-register tail: x becomes newest tap of new_state
                    nc.scalar.copy(st[:, HC + 3 : CW + 1 : 3], st[:, 0:HC])
                    nc.scalar.dma_start(
                        out=ns_d[r0 : r0 + P, c * SC : (c + 1) * SC],
                        in_=st[:, HC + 1 : CW + 1],
                    )
                    if bt == 0 and c + 1 < NHC:
                        bcast_chunk(c + 1)
                    nc.scalar.dma_start(
                        out=out_d[r0 : r0 + P, c * HC : (c + 1) * HC], in_=ot[:]
                    )

    nc.compile()
    return nc


def _get_program():
    if "nc" not in _cache:
        _cache["nc"] = _build_program()
    return _cache["nc"]


def _pack_weights(weight, bias):
    w = np.asarray(weight, dtype=np.float32)
    row = np.empty(WTOT, dtype=np.float32)
    for c in range(NHC):
        h0 = c * HC
        row[c * CW : c * CW + HC] = w[h0 : h0 + HC, 3]
        row[c * CW + HC : (c + 1) * CW] = w[h0 : h0 + HC, 0:3].reshape(-1)
    row[NHC * CW :] = np.asarray(bias, dtype=np.float32)
    return row.reshape(1, WTOT)


def run(x, state, weight, bias, trace=False, **spmd_kwargs):
    nc = _get_program()

    state_f = np.ascontiguousarray(state, dtype=np.float32).reshape(B, 3 * H)
    x = np.ascontiguousarray(x, dtype=np.float32)
    wrow = _pack_weights(weight, bias)
    onehot = np.zeros((WR, WR * P), dtype=np.float32)
    for r in range(WR):
        onehot[r, r * P : (r + 1) * P] = 1.0

    in_maps = [
        {
            "x": x[i * BS : (i + 1) * BS],
            "state": state_f[i * BS : (i + 1) * BS],
            "wrow": wrow,
            "onehot": onehot,
        }
        for i in range(NCORES)
    ]
    res = run_bass_kernel_spmd(
        nc, in_maps, list(range(NCORES)), trace=trace, **spmd_kwargs
    )
    out = np.concatenate([res.results[i]["out"] for i in range(NCORES)], axis=0)
    new_state = np.concatenate(
        [res.results[i]["new_state"] for i in range(NCORES)], axis=0
    ).reshape(B, H, K - 1)
    return (out, new_state), res


def kernel(x, state, weight, bias):
    (out, new_state), _ = run(x, state, weight, bias, trace=False)
    return out, new_state
